# revision 43
# baseline (speedup 1.0000x reference)
"""DeepSeek-V3 MoE block on 8 trn2 NeuronCores.

Expert-parallel sparse MoE, bf16 datapath (fp32 PSUM accumulation):
  - host computes routing + combine weights in fp32 numpy (dispatch
    metadata, bit-matched to the reference's sigmoid/top-k math)
  - token load is balanced by splitting oversized experts into near-equal
    parts: the 8x5 (core x slot) grid of cells holds 40 expert-parts, slot
    capacity P[k] = that tier's max part size, so every core runs the
    identical program (SPMD); cells of a split expert re-load its weights
  - per-cell inputs (gate/up weights, gathered tokens, down weights) are
    host-packed h-block-major into bf16 DRAM tensors, moved by ~1MB DMAs
    emitted in exact consumption order on the sync HWDGE ring so the PE
    streams right behind the DMA engines; the shared expert (best
    compute-per-byte) streams first while the slot weights arrive
  - device: per-expert gated MLP (bf16 matmuls, AF.Silu), down-projection
    computed transposed (out = [H, tokens]) so each slot's result leaves
    in a single packed DMA; combine-scaling happens in the host
    scatter-add; shared-expert MLP sharded 8-way on the intermediate dim
  - a burst of memset-fed dummy matmuls at kernel start warms the PE
    clock gate (HAM) while the first inputs stream in
"""

import os
import sys

sys.path.insert(0, "/opt/trn_rl_repo")

import numpy as np
import ml_dtypes

import concourse.bacc as bacc
import concourse.bass as bass
import concourse.mybir as mybir
import concourse.tile as tile
from concourse.bass_utils import run_bass_kernel_spmd

F32 = mybir.dt.float32
BF16 = mybir.dt.bfloat16
AF = mybir.ActivationFunctionType

BF = ml_dtypes.bfloat16

T, H, I, IS, E = 1024, 1024, 512, 2048, 32
G, TOPK_GROUP, TOP_K = 8, 4, 8
SCALE = 2.5
NCORES = 8
S = 5                    # expert-part slots per core
ISH = IS // NCORES       # shared-expert intermediate shard
P128 = 128
HT = H // P128           # 8 h-tiles
NWARM = 72               # dummy matmuls to warm the PE clock gate

LAST_RESULTS = None      # BassKernelResults of the most recent run


def _install_ntff_hook():
    """Provide antenv.axon_hooks + the ctypes NTFF profile hook when the
    container image lacks them (needed only for trace=True)."""
    import contextlib
    import ctypes
    import types

    try:
        from antenv.axon_hooks import get_axon_ntff_profile_hook  # noqa: F401
        return True
    except ImportError:
        pass
    try:
        import antenv
        so_path = "/opt/axon/libaxon_pjrt.so"
        lib = ctypes.CDLL(so_path)
        if not hasattr(lib, "axon_start_nrt_profile"):
            return False
        lib.axon_start_nrt_profile.argtypes = [
            ctypes.POINTER(ctypes.c_int64), ctypes.c_size_t]
        lib.axon_start_nrt_profile.restype = ctypes.c_int64
        lib.axon_stop_nrt_profile.argtypes = [ctypes.c_char_p]
        lib.axon_stop_nrt_profile.restype = ctypes.c_int64

        @contextlib.contextmanager
        def _hook(output_dir, device_ids):
            import jax
            jax.devices()
            if device_ids:
                ids = (ctypes.c_int64 * len(device_ids))(*device_ids)
                rc = lib.axon_start_nrt_profile(ids, len(device_ids))
            else:
                rc = lib.axon_start_nrt_profile(None, 0)
            if rc != 0:
                raise RuntimeError(f"axon_start_nrt_profile rc={rc}")
            try:
                yield
            finally:
                n = lib.axon_stop_nrt_profile(str(output_dir).encode())
                print(f"ntff profile: {n} file(s) -> {output_dir}",
                      file=sys.stderr)

        mod = types.ModuleType("antenv.axon_hooks")
        _state = {"hook": _hook}
        mod.set_axon_ntff_profile_hook = lambda h: _state.__setitem__("hook", h)
        mod.get_axon_ntff_profile_hook = lambda: _state["hook"]
        sys.modules["antenv.axon_hooks"] = mod
        antenv.axon_hooks = mod
        return True
    except Exception:
        return False


def _host_routing(x, gate_w, e_bias):
    """fp32 numpy mirror of reference._routing_combine.

    Returns (emask [T,E] bool, comb [T,E] fp32 combine weights)."""
    logits = x.astype(np.float32) @ gate_w.T.astype(np.float32)
    scores = 1.0 / (1.0 + np.exp(-logits))
    swb = scores + e_bias[None, :]
    swb_g = swb.reshape(T, G, E // G)
    gs = np.sort(swb_g, axis=-1)[..., -2:].sum(-1)          # top-2 sum per group
    thr4 = np.sort(gs, axis=-1)[:, -TOPK_GROUP][:, None]
    gmask = (gs >= thr4).astype(np.float32)
    smask = np.repeat(gmask, E // G, axis=-1)
    masked = swb * smask
    thr8 = np.sort(masked, axis=-1)[:, -TOP_K][:, None]
    emask = masked >= thr8
    sc = scores * emask
    comb = sc / (sc.sum(-1, keepdims=True) + 1e-20) * SCALE
    return emask, comb


def _pad4(n):
    return max(16, ((int(n) + 3) // 4) * 4)


def _chunks(p, limit=512):
    out = []
    o = 0
    while o < p:
        w = min(limit, p - o)
        out.append((o, w))
        o += w
    return out


def _split_cells(counts, tok_lists):
    """Balance 32 experts into NCORES*S cells by splitting big experts into
    near-equal parts. Returns cells: list of (expert, tok_array) sorted by
    descending part size, padded with (None, []) to exactly NCORES*S."""
    ncell = NCORES * S
    # smallest capacity C with sum(ceil(c/C)) <= ncell
    lo, hi = 1, int(max(counts))
    while lo < hi:
        mid = (lo + hi) // 2
        if sum(-(-int(c) // mid) for c in counts if c > 0) <= ncell:
            hi = mid
        else:
            lo = mid + 1
    C = lo
    cells = []
    for e in range(E):
        toks = tok_lists[e]
        n = len(toks)
        parts = max(1, -(-n // C))
        for j in range(parts):
            cells.append((e, toks[(j * n) // parts:((j + 1) * n) // parts]))
    cells.sort(key=lambda c: -len(c[1]))
    while len(cells) < ncell:
        cells.append((None, np.zeros((0,), dtype=np.int64)))
    return cells[:ncell]


def _build_program(P):
    """Emit the SPMD Bass program for slot capacities P (list of S ints)."""
    nc = bacc.Bacc(target_bir_lowering=False, debug=False)

    # per-slot inputs: gu = 8 h-blocks of [wg_h(512) | wu_h(512)];
    # xe = 8 h-blocks of [128, P]; wd = 4 i-blocks of [128, 1024]
    gu_d = [nc.dram_tensor(f"gu{k}", [P128, HT * 2 * I], BF16,
                           kind="ExternalInput") for k in range(S)]
    xe_d = [nc.dram_tensor(f"xe{k}", [P128, HT * P[k]], BF16,
                           kind="ExternalInput") for k in range(S)]
    wd_d = [nc.dram_tensor(f"wd{k}", [P128, (I // P128) * H], BF16,
                           kind="ExternalInput") for k in range(S)]
    wsgu_d = nc.dram_tensor("wsgu", [P128, HT * 2 * ISH], BF16,
                            kind="ExternalInput")
    xt_d = nc.dram_tensor("xt", [P128, HT * T], BF16, kind="ExternalInput")
    wsd_d = nc.dram_tensor("wsd", [P128, (ISH // P128) * H], BF16,
                           kind="ExternalInput")
    # outputs transposed: [128, 8*P] h-major blocks (columns = tokens)
    ro_d = [nc.dram_tensor(f"ro{k}", [P128, HT * P[k]], BF16,
                           kind="ExternalOutput") for k in range(S)]
    so_d = nc.dram_tensor("so", [P128, HT * T], BF16, kind="ExternalOutput")

    NII = I // P128       # 4 expert i-tiles
    NIS = ISH // P128     # 2 shared i-tiles

    with tile.TileContext(nc) as tc:
        with (
            tc.tile_pool(name="const", bufs=1) as cpool,
            tc.tile_pool(name="acts", bufs=2) as apool,
            tc.tile_pool(name="stage", bufs=3) as stpool,
            tc.tile_pool(name="ps", bufs=8, space="PSUM") as ps,
        ):
            # ---- input DMAs, in consumption order (HWDGE FIFO ring).
            # Uniform ~0.25-1MB chunks keep per-chunk completion latency low
            # so consumers never wait on a half-delivered multi-MB block.
            # warmup weights come from memset — no DMA dependency
            wk_t = cpool.tile([P128, P128], BF16, tag="wk")
            nc.vector.memset(wk_t[:], 0.0)

            # shared-expert gate/up inputs first: best compute-per-byte,
            # consumable per xt h-block while the slot weights stream in
            wsgu_t = cpool.tile([P128, HT * 2 * ISH], BF16, tag="wsgu")
            xt_t = cpool.tile([P128, HT * T], BF16, tag="xt")
            BW = 2 * ISH          # wsgu h-block width
            # tiny first chunks: the first shared-gu h-steps become runnable
            # while the DMA stream is still ramping up
            # first chunks ride the scalar HWDGE ring in parallel with the
            # sync ring to overlap per-DMA fixed costs during the ramp
            nc.scalar.dma_start(out=wsgu_t[:, :BW], in_=wsgu_d[:, :BW])
            nc.scalar.dma_start(out=xt_t[:, :T], in_=xt_d[:, :T])
            for wsl, xsl in (((1, 2), (1, 2)),
                             ((2, 4), (2, 4)), ((4, 8), (4, 6)),
                             (None, (6, 8))):
                if wsl is not None:
                    nc.sync.dma_start(
                        out=wsgu_t[:, wsl[0] * BW:wsl[1] * BW],
                        in_=wsgu_d[:, wsl[0] * BW:wsl[1] * BW])
                nc.sync.dma_start(out=xt_t[:, xsl[0] * T:xsl[1] * T],
                                  in_=xt_d[:, xsl[0] * T:xsl[1] * T])

            gu_t, xe_t, wd_t = [], [], []

            def slot_dmas(k):
                g = cpool.tile([P128, HT * 2 * I], BF16, tag="gu", bufs=3,
                               name=f"gu{k}")
                x = cpool.tile([P128, HT * P[k]], BF16, tag="xe", bufs=3,
                               name=f"xe{k}")
                w = cpool.tile([P128, (I // P128) * H], BF16, tag="wd",
                               bufs=3, name=f"wd{k}")
                gu_t.append(g); xe_t.append(x); wd_t.append(w)
                nc.sync.dma_start(out=g[:, :HT * I], in_=gu_d[k][:, :HT * I])
                nc.sync.dma_start(out=g[:, HT * I:], in_=gu_d[k][:, HT * I:])
                nc.sync.dma_start(out=x[:], in_=xe_d[k][:])
                nc.sync.dma_start(out=w[:], in_=wd_d[k][:])

            slot_dmas(0)
            slot_dmas(1)
            slot_dmas(2)
            wsd_t = cpool.tile([P128, NIS * H], BF16, tag="wsd")
            nc.sync.dma_start(out=wsd_t[:], in_=wsd_d[:])
            for k in range(3, S):
                slot_dmas(k)

            # ---- PE clock-gate warmup: dummy matmuls, result discarded ----
            wps = ps.tile([P128, P128], F32, tag="acc", name="warm")
            for _ in range(NWARM):
                nc.tensor.matmul(wps[:], lhsT=wk_t[:], rhs=wk_t[:],
                                 start=True, stop=True)

            def gated_mlp(k):
                g, x, w = gu_t[k], xe_t[k], wd_t[k]
                pk = P[k]
                acts = [apool.tile([P128, pk], BF16, tag="acts", bufs=6,
                                   name=f"a{ii}") for ii in range(NII)]
                for ii in range(NII):
                    for (mo, mw) in _chunks(pk):
                        h1 = ps.tile([P128, mw], F32, tag="acc", name="h1")
                        h2 = ps.tile([P128, mw], F32, tag="acc", name="h2")
                        for h in range(HT):
                            o = h * 2 * I
                            nc.tensor.matmul(
                                h1[:],
                                lhsT=g[:, o + ii * P128:o + (ii + 1) * P128],
                                rhs=x[:, h * pk + mo:h * pk + mo + mw],
                                start=(h == 0), stop=(h == HT - 1))
                        for h in range(HT):
                            o = h * 2 * I
                            nc.tensor.matmul(
                                h2[:],
                                lhsT=g[:, o + I + ii * P128:
                                       o + I + (ii + 1) * P128],
                                rhs=x[:, h * pk + mo:h * pk + mo + mw],
                                start=(h == 0), stop=(h == HT - 1))
                        sl = stpool.tile([P128, mw], F32, tag="sl", name="sl")
                        nc.scalar.activation(sl[:], h1[:], AF.Silu)
                        nc.vector.tensor_mul(acts[ii][:, mo:mo + mw],
                                             sl[:], h2[:])

                # transposed down-proj: out block hb = [128 h, pk tokens]
                ost = stpool.tile([P128, HT * pk], BF16, tag="ost", bufs=2,
                                  name="ost")
                hw = (HT // 2) * pk
                for hb in range(HT):
                    for (mo, mw) in _chunks(pk):
                        dps = ps.tile([P128, mw], F32, tag="acc", name="dps")
                        for ii in range(NII):
                            nc.tensor.matmul(
                                dps[:],
                                lhsT=w[:, ii * H + hb * P128:
                                       ii * H + (hb + 1) * P128],
                                rhs=acts[ii][:, mo:mo + mw],
                                start=(ii == 0), stop=(ii == NII - 1))
                        # alternate engines so trailing copies don't
                        # serialize on one queue behind the last matmul
                        if hb % 2 == 0:
                            nc.scalar.activation(
                                ost[:, hb * pk + mo:hb * pk + mo + mw],
                                dps[:], AF.Copy)
                        else:
                            nc.vector.tensor_copy(
                                ost[:, hb * pk + mo:hb * pk + mo + mw],
                                dps[:])
                    # last slot: stream out on the scalar HWDGE ring as
                    # blocks complete so the final transfer is tiny
                    if k == S - 1 and hb == HT // 2 - 1:
                        nc.scalar.dma_start(out=ro_d[k][:, :hw],
                                            in_=ost[:, :hw])
                    if k == S - 1 and hb == HT - 2:
                        nc.scalar.dma_start(
                            out=ro_d[k][:, hw:(HT - 1) * pk],
                            in_=ost[:, hw:(HT - 1) * pk])
                if k == S - 1:
                    nc.scalar.dma_start(out=ro_d[k][:, (HT - 1) * pk:],
                                        in_=ost[:, (HT - 1) * pk:])
                else:
                    nc.gpsimd.dma_start(out=ro_d[k][:], in_=ost[:])

            acts_s = [apool.tile([P128, T], BF16, tag="acts_s", bufs=2,
                                 name=f"as{ii}") for ii in range(NIS)]

            def shared_gu():
                # h-major: consume xt h-blocks as they land, 8 PSUM banks
                # hold the (ii, chunk, gate/up) accumulators
                chs = _chunks(T)
                h1 = [[ps.tile([P128, mw], F32, tag="acc", name=f"sg{ii}{ci}")
                       for ci, (mo, mw) in enumerate(chs)]
                      for ii in range(NIS)]
                h2 = [[ps.tile([P128, mw], F32, tag="acc", name=f"su{ii}{ci}")
                       for ci, (mo, mw) in enumerate(chs)]
                      for ii in range(NIS)]
                for h in range(HT):
                    o = h * 2 * ISH
                    for ii in range(NIS):
                        for ci, (mo, mw) in enumerate(chs):
                            nc.tensor.matmul(
                                h1[ii][ci][:],
                                lhsT=wsgu_t[:, o + ii * P128:
                                            o + (ii + 1) * P128],
                                rhs=xt_t[:, h * T + mo:h * T + mo + mw],
                                start=(h == 0), stop=(h == HT - 1))
                            nc.tensor.matmul(
                                h2[ii][ci][:],
                                lhsT=wsgu_t[:, o + ISH + ii * P128:
                                            o + ISH + (ii + 1) * P128],
                                rhs=xt_t[:, h * T + mo:h * T + mo + mw],
                                start=(h == 0), stop=(h == HT - 1))
                for ii in range(NIS):
                    for ci, (mo, mw) in enumerate(chs):
                        sl = stpool.tile([P128, mw], F32, tag="sl", name="sl")
                        nc.scalar.activation(sl[:], h1[ii][ci][:], AF.Silu)
                        nc.vector.tensor_mul(acts_s[ii][:, mo:mo + mw],
                                             sl[:], h2[ii][ci][:])

            def shared_down():
                for half in range(2):
                    ost = stpool.tile([P128, HT * T // 2], BF16, tag="osts",
                                      bufs=2, name="osts")
                    for hb in range(HT // 2):
                        hbb = half * (HT // 2) + hb
                        for (mo, mw) in _chunks(T):
                            dps = ps.tile([P128, mw], F32, tag="acc",
                                          name="dps")
                            for ii in range(NIS):
                                nc.tensor.matmul(
                                    dps[:],
                                    lhsT=wsd_t[:, ii * H + hbb * P128:
                                               ii * H + (hbb + 1) * P128],
                                    rhs=acts_s[ii][:, mo:mo + mw],
                                    start=(ii == 0), stop=(ii == NIS - 1))
                            nc.vector.tensor_copy(
                                ost[:, hb * T + mo:hb * T + mo + mw], dps[:])
                    nc.gpsimd.dma_start(
                        out=so_d[:, half * (HT * T // 2):
                                 (half + 1) * (HT * T // 2)],
                        in_=ost[:])

            shared_gu()
            gated_mlp(0)
            gated_mlp(1)
            gated_mlp(2)
            shared_down()
            for k in range(3, S):
                gated_mlp(k)

    nc.compile()
    return nc


def _prepare(inputs):
    """Host-side dispatch prep: returns (in_maps, P, cells)."""
    x = np.ascontiguousarray(inputs["hidden_states"], dtype=np.float32)
    gate_w = np.asarray(inputs["gate_w"], dtype=np.float32)
    e_bias = np.asarray(inputs["e_bias"], dtype=np.float32)
    w_gate = np.asarray(inputs["w_gate"], dtype=np.float32)
    w_up = np.asarray(inputs["w_up"], dtype=np.float32)
    w_down = np.asarray(inputs["w_down"], dtype=np.float32)
    ws_gate = np.asarray(inputs["ws_gate"], dtype=np.float32)
    ws_up = np.asarray(inputs["ws_up"], dtype=np.float32)
    ws_down = np.asarray(inputs["ws_down"], dtype=np.float32)

    emask, comb = _host_routing(x, gate_w, e_bias)
    counts = emask.sum(0).astype(np.int64)
    tok_lists = [np.nonzero(emask[:, e])[0] for e in range(E)]
    cells = _split_cells(counts, tok_lists)     # len NCORES*S, sorted desc
    grid = [[cells[k * NCORES + c] for c in range(NCORES)] for k in range(S)]
    P = [_pad4(max(len(cell[1]) for cell in tier)) for tier in grid]

    xb = x.astype(BF)
    wgb = w_gate.astype(BF)
    wub = w_up.astype(BF)
    wdb = w_down.astype(BF)

    xt = np.empty((P128, HT * T), dtype=BF)
    for h in range(HT):
        xt[:, h * T:(h + 1) * T] = xb[:, h * P128:(h + 1) * P128].T

    in_maps = []
    for c in range(NCORES):
        wsgu = np.empty((P128, HT * 2 * ISH), dtype=BF)
        for h in range(HT):
            o = h * 2 * ISH
            wsgu[:, o:o + ISH] = \
                ws_gate[h * P128:(h + 1) * P128, c * ISH:(c + 1) * ISH]
            wsgu[:, o + ISH:o + 2 * ISH] = \
                ws_up[h * P128:(h + 1) * P128, c * ISH:(c + 1) * ISH]
        wsd = np.empty((P128, (ISH // P128) * H), dtype=BF)
        for ii in range(ISH // P128):
            wsd[:, ii * H:(ii + 1) * H] = \
                ws_down[c * ISH + ii * P128:c * ISH + (ii + 1) * P128, :]
        m = {"xt": xt, "wsgu": wsgu, "wsd": wsd}

        for k in range(S):
            e, toks = grid[k][c]
            n = len(toks)
            gu = np.zeros((P128, HT * 2 * I), dtype=BF)
            xp = np.zeros((P128, HT * P[k]), dtype=BF)
            wd = np.zeros((P128, (I // P128) * H), dtype=BF)
            if e is not None:
                xe = xb[toks].T                    # [H, n]
                for h in range(HT):
                    o = h * 2 * I
                    gu[:, o:o + I] = wgb[e, h * P128:(h + 1) * P128, :]
                    gu[:, o + I:o + 2 * I] = wub[e, h * P128:(h + 1) * P128, :]
                    if n:
                        xp[:, h * P[k]:h * P[k] + n] = \
                            xe[h * P128:(h + 1) * P128, :]
                for ii in range(I // P128):
                    wd[:, ii * H:(ii + 1) * H] = \
                        wdb[e, ii * P128:(ii + 1) * P128, :]
            m[f"gu{k}"] = gu
            m[f"xe{k}"] = xp
            m[f"wd{k}"] = wd
        in_maps.append(m)

    return in_maps, P, grid, comb


def _recombine(results, P, grid, comb):
    out = np.zeros((T, H), dtype=np.float32)
    # shared partials: so[p, h*T + t] = partial[t, h*128+p]
    for c in range(NCORES):
        so = np.asarray(results[c]["so"], dtype=np.float32)
        out += so.reshape(P128, HT, T).transpose(2, 1, 0).reshape(T, H)
    # routed: ro[p, hb*P + j] = down_out[token j, hb*128+p]; scale on host
    for c in range(NCORES):
        for k in range(S):
            e, toks = grid[k][c]
            n = len(toks)
            if e is None or n == 0:
                continue
            ro = np.asarray(results[c][f"ro{k}"], dtype=np.float32)
            contrib = ro.reshape(P128, HT, P[k])[:, :, :n]   # [128, HT, n]
            contrib = contrib.transpose(2, 1, 0).reshape(n, H)
            out[toks] += contrib * comb[toks, e][:, None]
    return out


def kernel(**inputs):
    global LAST_RESULTS
    in_maps, P, grid, comb = _prepare(inputs)
    nc = _build_program(P)
    trace = bool(int(os.environ.get("KERNEL_TRACE", "0")))
    if trace:
        trace = _install_ntff_hook()
    LAST_RESULTS = run_bass_kernel_spmd(
        nc, in_maps, list(range(NCORES)), trace=trace)
    results = LAST_RESULTS.results
    return _recombine(results, P, grid, comb)
